# revision 1
# baseline (speedup 1.0000x reference)
"""Trainium2 Bass kernel for nn_DSCBR (gnn_message_passing).

Strategy (8 NeuronCores, SPMD):
- Node tables padded per-core (rows multiple of 128); dest-row sharding.
- SpMM = dma_gather of source rows (int16-windowed) + per-128-edge-chunk
  selection-matrix matmul (TensorE) accumulating dest windows in PSUM,
  drained into an SBUF accumulator; all-gather of the new feature table
  between layers (collectives overlap the other graph's compute).
- Losses computed batch-sharded (256 rows/core) + tiny AllReduce.
"""
import os
import sys
import types

sys.path.insert(0, "/opt/trn_rl_repo")

import numpy as np

import concourse.bass as bass
import concourse.bacc as bacc
import concourse.mybir as mybir
import concourse.tile as tile
from concourse.bass_utils import run_bass_kernel_spmd
from concourse.masks import make_identity

P = 128
NCORES = 8
SRC_WIN = 32768
GI_MAX = 2048
D = 64
NU, NI, NB = 100000, 50000, 20000
BATCH = 2048
F32 = mybir.dt.float32
I32 = mybir.dt.int32
I16 = mybir.dt.int16
AF = mybir.ActivationFunctionType
ALU = mybir.AluOpType


# ---------------------------------------------------------------- host prep

def node_map(orig_n, ncores):
    # round-robin: orig row r -> core r%ncores, local slot r//ncores
    per = orig_n // ncores
    assert per * ncores == orig_n
    R = ((per + 2 * P - 1) // (2 * P)) * (2 * P)   # multiple of 256 (win pairs)
    V = R * ncores
    def mapr(r):
        return (r % ncores) * R + (r // ncores)
    return per, R, V, mapr


def build_graph_stream(rows_mapped, cols_mapped, vals, R, V_src, ncores):
    """Win-pair (256-dest-row) grouping. Returns
    (idx_streams [ncores, tch*128] i16, lrowA/lrowB [ncores, tch*128] f32,
     val_streams [ncores, tch*128] f32, program, tch).
    program: [(s, batches)]; batch = [(wp, nchunks_here, first, last)]."""
    nwp = R // (2 * P)
    nsrc = (V_src + SRC_WIN - 1) // SRC_WIN
    core_of = rows_mapped // R
    dest_local = rows_mapped % R
    wp = dest_local // (2 * P)
    lrow = dest_local % (2 * P)            # 0..255 within pair
    swin = cols_mapped // SRC_WIN
    sidx = cols_mapped % SRC_WIN

    counts = np.zeros((ncores, nsrc, nwp), np.int64)
    order = np.lexsort((lrow, wp, swin, core_of))
    r_s, w_s, l_s, si_s, v_s, c_s = (wp[order], swin[order], lrow[order],
                                     sidx[order], vals[order], core_of[order])
    np.add.at(counts, (c_s, w_s, r_s), 1)
    nchunks = (np.max(counts, axis=0) + P - 1) // P
    for w in range(nwp):
        if nchunks[:, w].sum() == 0:
            nchunks[0, w] = 1

    key = c_s * (nsrc * nwp) + w_s * nwp + r_s
    starts = np.searchsorted(key, np.arange(ncores * nsrc * nwp))
    ends = np.searchsorted(key, np.arange(ncores * nsrc * nwp) + 1)

    total_chunks = int(nchunks.sum())
    idx_streams = np.zeros((ncores, total_chunks * P), np.int16)
    lrowA = np.full((ncores, total_chunks * P), 300.0, np.float32)
    lrowB = np.full((ncores, total_chunks * P), 300.0, np.float32)
    val_streams = np.zeros((ncores, total_chunks * P), np.float32)
    for c in range(ncores):
        pos = 0
        for s in range(nsrc):
            for w in range(nwp):
                nc_ = int(nchunks[s, w])
                if nc_ == 0:
                    continue
                k = c * (nsrc * nwp) + s * nwp + w
                a, b = starts[k], ends[k]
                n = b - a
                idx_streams[c, pos:pos + n] = si_s[a:b]
                lr = l_s[a:b]
                lrowA[c, pos:pos + n] = np.where(lr < P, lr, 300.0)
                lrowB[c, pos:pos + n] = np.where(lr >= P, lr - P, 300.0)
                val_streams[c, pos:pos + n] = v_s[a:b]
                pos += nc_ * P
        assert pos == total_chunks * P

    program = []
    for s in range(nsrc):
        wins = [(w, int(nchunks[s, w])) for w in range(nwp) if nchunks[s, w] > 0]
        batches = []
        cur, cur_n = [], 0
        for w, ncw in wins:
            done = 0
            while done < ncw:
                room = (GI_MAX // P) - cur_n
                if room == 0:
                    batches.append(cur)
                    cur, cur_n = [], 0
                    room = GI_MAX // P
                take = min(room, ncw - done)
                cur.append((w, take, done == 0, done + take == ncw))
                cur_n += take
                done += take
        if cur:
            batches.append(cur)
        program.append((s, batches))
    return idx_streams, lrowA, lrowB, val_streams, program, total_chunks


def wrap_idx16(flat):
    # index i -> partition i%16, col i//16; replicated x8 down partitions
    return np.ascontiguousarray(np.tile(flat.reshape(-1, 16).T.astype(np.int16), (8, 1)))


def idx_cols_i32(flat):
    # [n] -> [128, n/128] int32; col k = rows [128k, 128k+128)
    n = flat.shape[0]
    assert n % P == 0
    return np.ascontiguousarray(flat.reshape(-1, P).T.astype(np.int32))


def preprocess(inputs, ncores=NCORES):
    u = np.asarray(inputs["users_feature"], np.float32)
    it = np.asarray(inputs["items_feature"], np.float32)
    b = np.asarray(inputs["bundles_feature"], np.float32)

    per1, R1, V1, map1 = node_map(NU + NI, ncores)
    per2, R2, V2, map2 = node_map(NU + NB, ncores)
    perb, RB, VB, mapb = node_map(NB, ncores)

    f0_il = np.zeros((V1, D), np.float32)
    f0_il[map1(np.arange(NU + NI))] = np.concatenate([u, it], 0)
    f0_bl = np.zeros((V2, D), np.float32)
    f0_bl[map2(np.arange(NU + NB))] = np.concatenate([u, b], 0)

    il = build_graph_stream(map1(np.asarray(inputs["il_row"])),
                            map1(np.asarray(inputs["il_col"])),
                            np.asarray(inputs["il_val"], np.float32), R1, V1, ncores)
    bl = build_graph_stream(map2(np.asarray(inputs["bl_row"])),
                            map2(np.asarray(inputs["bl_col"])),
                            np.asarray(inputs["bl_val"], np.float32), R2, V2, ncores)
    ag = build_graph_stream(mapb(np.asarray(inputs["agg_row"])),
                            map1(np.asarray(inputs["agg_col"]) + NU),
                            np.asarray(inputs["agg_val"], np.float32), RB, V1, ncores)

    users = np.asarray(inputs["users"]).astype(np.int64)
    bundles = np.asarray(inputs["bundles"]).astype(np.int64)
    loss = {}
    bsh = BATCH // ncores
    for c in range(ncores):
        sl = slice(c * bsh, (c + 1) * bsh)
        loss[c] = dict(
            u_il=idx_cols_i32(map1(users[sl])),
            u_bl=idx_cols_i32(map2(users[sl])),
            b_il0=idx_cols_i32(mapb(bundles[sl, 0])),
            b_il1=idx_cols_i32(mapb(bundles[sl, 1])),
            b_bl0=idx_cols_i32(map2(bundles[sl, 0] + NU)),
            b_bl1=idx_cols_i32(map2(bundles[sl, 1] + NU)),
        )
    aug_u_bl = idx_cols_i32(map2(users))
    aug_b0_bl = idx_cols_i32(map2(bundles[:, 0] + NU))
    aug_b0_il = idx_cols_i32(mapb(bundles[:, 0]))

    return dict(f0_il=f0_il, f0_bl=f0_bl, il=il, bl=bl, ag=ag,
                loss=loss, aug_u_bl=aug_u_bl, aug_b0_bl=aug_b0_bl, aug_b0_il=aug_b0_il,
                dims=(R1, V1, R2, V2, RB, VB))


# ---------------------------------------------------------------- bass build

class Ctx:
    pass


def emit_spmm(cx, name, prog_info, R, V_src, table_ap, idx_dram, lrA_sb, lrB_sb,
              valv_sb, raw_sb, scale_const):
    """One SpMM layer, win-pair groups, bf16 matmul path.
    raw_sb [128, nwin*64] f32 receives segment sums * scale_const."""
    nc = cx.nc
    nwp = R // (2 * P)
    program = prog_info[4]
    chunk_pos = 0
    wp_started = [False] * nwp
    BF = mybir.dt.bfloat16
    for s, batches in program:
        wrows = min(SRC_WIN, V_src - s * SRC_WIN)
        src_slice = table_ap[s * SRC_WIN: s * SRC_WIN + wrows, :]
        open_psum = {}
        for batch in batches:
            nch = sum(seg[1] for seg in batch)
            gi = nch * P
            idx_t = cx.idxp.tile([128, GI_MAX // 16], I16, tag="gidx", name="gidx")
            nc.sync.dma_start(out=idx_t[:, :gi // 16],
                              in_=idx_dram[:, chunk_pos * 8: chunk_pos * 8 + gi // 16])
            g = cx.gp.tile([P, (GI_MAX // P) * D], F32, tag="gg", name="gg")
            nc.gpsimd.dma_gather(
                out_ap=g[:, :nch * D].rearrange("p (c d) -> p c d", c=nch),
                in_ap=src_slice,
                idxs_ap=idx_t[:, :gi // 16],
                num_idxs=gi,
                num_idxs_reg=gi,
                elem_size=D,
                single_packet=False,
            )
            # val fold + cast to bf16 (one op per batch)
            gv = cx.gp.tile([P, (GI_MAX // P) * D], BF, tag="gv", name="gv")
            nc.vector.tensor_mul(
                gv[:, :nch * D].rearrange("p (c d) -> p c d", c=nch),
                g[:, :nch * D].rearrange("p (c d) -> p c d", c=nch),
                valv_sb[:, chunk_pos:chunk_pos + nch].to_broadcast([P, nch, D]),
            )
            # batched selection matrices (bf16): selX[p, c*128+j] = (lrowX[p,c] == j)
            selA = cx.selp.tile([P, (GI_MAX // P) * P], BF, tag="selA", name="selA")
            selB = cx.selp.tile([P, (GI_MAX // P) * P], BF, tag="selB", name="selB")
            iota_rep = cx.iota_bf[:].rearrange("p (o j) -> p o j", o=1).to_broadcast([P, nch, P])
            nc.vector.tensor_tensor(
                out=selA[:, :nch * P].rearrange("p (c j) -> p c j", c=nch),
                in0=iota_rep,
                in1=lrA_sb[:, chunk_pos:chunk_pos + nch].to_broadcast([P, nch, P]),
                op=ALU.is_equal)
            nc.vector.tensor_tensor(
                out=selB[:, :nch * P].rearrange("p (c j) -> p c j", c=nch),
                in0=iota_rep,
                in1=lrB_sb[:, chunk_pos:chunk_pos + nch].to_broadcast([P, nch, P]),
                op=ALU.is_equal)
            bc = 0
            for (w, ncw, first, last) in batch:
                if first:
                    pA = cx.psp.tile([P, D], F32, space="PSUM", tag="psegA", name="psegA", bufs=2)
                    pB = cx.psp.tile([P, D], F32, space="PSUM", tag="psegB", name="psegB", bufs=2)
                    open_psum[w] = (pA, pB)
                pA, pB = open_psum[w]
                for k in range(ncw):
                    c = bc + k
                    st = (first and k == 0)
                    sp = (last and k == ncw - 1)
                    nc.tensor.matmul(out=pA[:], lhsT=selA[:, c * P:(c + 1) * P],
                                     rhs=gv[:, c * D:(c + 1) * D], start=st, stop=sp)
                    nc.tensor.matmul(out=pB[:], lhsT=selB[:, c * P:(c + 1) * P],
                                     rhs=gv[:, c * D:(c + 1) * D], start=st, stop=sp)
                if last:
                    for half, pt in ((0, pA), (1, pB)):
                        wfull = 2 * w + half
                        dst = raw_sb[:, wfull * D:(wfull + 1) * D]
                        if scale_const == 1.0:
                            if not wp_started[w]:
                                nc.vector.tensor_copy(dst, pt[:])
                            else:
                                nc.vector.tensor_add(dst, dst, pt[:])
                        else:
                            if not wp_started[w]:
                                nc.vector.tensor_scalar_mul(dst, pt[:], scale_const)
                            else:
                                t = cx.selp.tile([P, D], F32, tag="segtmp", name="segtmp")
                                nc.vector.tensor_scalar_mul(t[:], pt[:], scale_const)
                                nc.vector.tensor_add(dst, dst, t[:])
                    wp_started[w] = True
                    del open_psum[w]
                bc += ncw
            chunk_pos += nch
    assert all(wp_started), f"{name}: some win-pairs never written"


def emit_epilogue(cx, raw_sb, acc_sb, nwin, layer_scale_inv):
    """acc += f_next / max(||f_next||,1e-12), where f_next = raw (already scaled).
    Processes windows in groups to bound SBUF temps."""
    nc = cx.nc
    EPG = 32
    for g0 in range(0, nwin, EPG):
        ng = min(EPG, nwin - g0)
        sl = slice(g0 * D, (g0 + ng) * D)
        sq = cx.ep.tile([P, EPG * D], F32, tag="ep_sq", name="ep_sq")
        nc.vector.tensor_mul(sq[:, :ng * D], raw_sb[:, sl], raw_sb[:, sl])
        ss = cx.ep.tile([P, EPG], F32, tag="ep_ss", name="ep_ss")
        nc.vector.reduce_sum(ss[:, :ng], sq[:, :ng * D].rearrange("p (w d) -> p w d", w=ng),
                             axis=mybir.AxisListType.X)
        snorm = cx.ep.tile([P, EPG], F32, tag="ep_sn", name="ep_sn")
        nc.scalar.activation(snorm[:, :ng], ss[:, :ng], AF.Sqrt)
        nc.vector.tensor_scalar_max(snorm[:, :ng], snorm[:, :ng], 1e-12)
        rn = cx.ep.tile([P, EPG], F32, tag="ep_rn", name="ep_rn")
        nc.vector.reciprocal(rn[:, :ng], snorm[:, :ng])
        contrib = cx.ep.tile([P, EPG * D], F32, tag="ep_ct", name="ep_ct")
        nc.vector.tensor_mul(
            contrib[:, :ng * D].rearrange("p (w d) -> p w d", w=ng),
            raw_sb[:, sl].rearrange("p (w d) -> p w d", w=ng),
            rn[:, :ng].to_broadcast([P, ng, D]),
        )
        nc.vector.tensor_add(acc_sb[:, sl], acc_sb[:, sl], contrib[:, :ng * D])


def indirect_gather_rows(cx, out_sb, table_ap, idx_sb, ncols):
    """out_sb[:, k*64:(k+1)*64] = table[idx_sb[:,k]] for k in range(ncols)."""
    nc = cx.nc
    for k in range(ncols):
        nc.gpsimd.indirect_dma_start(
            out=out_sb[:, k * D:(k + 1) * D],
            out_offset=None,
            in_=table_ap,
            in_offset=bass.IndirectOffsetOnAxis(ap=idx_sb[:, k:k + 1], axis=0),
        )


def normalize_rows(cx, x_sb, ngroups, tag):
    """Row-normalize [128, ngroups*64] (each 64-wide group a row): x /= max(||x||,1e-12)."""
    nc = cx.nc
    sq = cx.lp.tile([P, ngroups * D], F32, tag=f"{tag}_sq")
    nc.vector.tensor_mul(sq[:], x_sb[:, :ngroups * D], x_sb[:, :ngroups * D])
    ss = cx.lp.tile([P, ngroups], F32, tag=f"{tag}_ss")
    nc.vector.reduce_sum(ss[:], sq[:].rearrange("p (w d) -> p w d", w=ngroups),
                         axis=mybir.AxisListType.X)
    sn = cx.lp.tile([P, ngroups], F32, tag=f"{tag}_sn")
    nc.scalar.activation(sn[:], ss[:], AF.Sqrt)
    nc.vector.tensor_scalar_max(sn[:], sn[:], 1e-12)
    rn = cx.lp.tile([P, ngroups], F32, tag=f"{tag}_rn")
    nc.vector.reciprocal(rn[:], sn[:])
    nc.vector.tensor_mul(
        x_sb[:, :ngroups * D].rearrange("p (w d) -> p w d", w=ngroups),
        x_sb[:, :ngroups * D].rearrange("p (w d) -> p w d", w=ngroups),
        rn[:].to_broadcast([P, ngroups, D]),
    )


def rowdot(cx, a_sb, b_sb, out_sb, ngroups, tag):
    """out[p, g] = sum_d a[p, g*64+d]*b[p, g*64+d]."""
    nc = cx.nc
    t = cx.lp.tile([P, ngroups * D], F32, tag=f"{tag}_t")
    nc.vector.tensor_mul(t[:], a_sb[:, :ngroups * D], b_sb[:, :ngroups * D])
    nc.vector.reduce_sum(out_sb[:, :ngroups], t[:].rearrange("p (w d) -> p w d", w=ngroups),
                         axis=mybir.AxisListType.X)


def transpose_groups(cx, src_sb, ngroups, tag):
    """[128, ngroups*64] (row r of group g at [r, g*64:]) -> [64, ngroups*128] T."""
    nc = cx.nc
    out = cx.lp.tile([P, ngroups * P], F32, tag=f"{tag}_T")
    for g in range(ngroups):
        pt = cx.psp.tile([P, P], F32, space="PSUM", tag="tr_ps", bufs=1)
        nc.tensor.transpose(out=pt[:D, :P], in_=src_sb[:, g * D:(g + 1) * D],
                            identity=cx.ident[:])
        nc.vector.tensor_copy(out[:D, g * P:(g + 1) * P], pt[:D, :P])
    return out


def build(pp, phases=("il", "bl", "ag", "loss"), debug_tables=False):
    R1, V1, R2, V2, RB, VB = pp["dims"]
    nw1, nw2, nwb = R1 // P, R2 // P, RB // P
    nc = bacc.Bacc("TRN2", target_bir_lowering=False, debug=False, num_devices=NCORES)
    cx = Ctx()
    cx.nc = nc

    # ---- dram inputs
    f0_il = nc.dram_tensor("f0_il", [V1, D], F32, kind="ExternalInput")
    f0_bl = nc.dram_tensor("f0_bl", [V2, D], F32, kind="ExternalInput")
    f0_il_sh = nc.dram_tensor("f0_il_sh", [R1, D], F32, kind="ExternalInput")
    f0_bl_sh = nc.dram_tensor("f0_bl_sh", [R2, D], F32, kind="ExternalInput")
    g_in = {}
    BF = mybir.dt.bfloat16
    for gname, st in (("il", pp["il"]), ("bl", pp["bl"]), ("ag", pp["ag"])):
        tch = st[5]
        g_in[gname] = dict(
            idx=nc.dram_tensor(f"{gname}_idx", [128, tch * 8], I16, kind="ExternalInput"),
            lrowA=nc.dram_tensor(f"{gname}_lrowA", [128, tch], BF, kind="ExternalInput"),
            lrowB=nc.dram_tensor(f"{gname}_lrowB", [128, tch], BF, kind="ExternalInput"),
            val=nc.dram_tensor(f"{gname}_val", [128, tch], F32, kind="ExternalInput"),
            tch=tch,
        )
    lidx = {k: nc.dram_tensor(f"loss_{k}", [128, v.shape[1]], I32, kind="ExternalInput")
            for k, v in pp["loss"][0].items()}
    aug_in = {k: nc.dram_tensor(k, [128, 16], I32, kind="ExternalInput")
              for k in ("aug_u_bl", "aug_b0_bl", "aug_b0_il")}
    out_t = nc.dram_tensor("out", [1, 2], F32, kind="ExternalOutput")
    dbg = {}

    with tile.TileContext(nc) as tc:
        cx.tc = tc
        es = []
        def pool(name, bufs, **kw):
            p = tc.tile_pool(name=name, bufs=bufs, **kw)
            es.append(p)
            return p.__enter__()
        cx.psp = pool("psum", 4, space="PSUM")
        cx.dramp = pool("dram", 1, space="DRAM")
        cx.cp = pool("const", 1)

        # constants (persist through loss phase)
        iota_i = cx.cp.tile([P, P], I32)
        nc.gpsimd.iota(iota_i[:], pattern=[[1, P]], base=0, channel_multiplier=0)
        cx.iota_f = cx.cp.tile([P, P], F32)
        nc.vector.tensor_copy(cx.iota_f[:], iota_i[:])
        cx.iota_bf = cx.cp.tile([P, P], mybir.dt.bfloat16)
        nc.vector.tensor_copy(cx.iota_bf[:], iota_i[:])
        cx.ident = cx.cp.tile([P, P], F32)
        make_identity(nc, cx.ident[:])
        ones_col = cx.cp.tile([P, 1], F32)
        nc.vector.memset(ones_col[:], 1.0)

        def ag_tiles(Rr, Vv, nm):
            ain = cx.dramp.tile([Rr, D], F32, tag=f"{nm}_agin", name=f"{nm}_agin")
            aout = cx.dramp.tile([Vv, D], F32, addr_space="Shared", tag=f"{nm}_agout",
                                 name=f"{nm}_agout")
            return ain, aout

        acc_il_full = acc_bl_full = ilb_full = None

        # ---------- SpMM phases in a scoped pool block ----------
        es2 = []
        def pool2(name, bufs, **kw):
            p = tc.tile_pool(name=name, bufs=bufs, **kw)
            es2.append(p)
            return p.__enter__()
        cx.gp = pool2("gather", 2)
        cx.idxp = pool2("gidx", 3)
        cx.selp = pool2("sel", 2)
        cx.ep = pool2("epil", 1)
        cx.mp = pool2("meta", 1)
        cx.accp = pool2("accs", 1)

        meta_sb = {}
        for gname in ("il", "bl", "ag"):
            if (gname in phases) or (gname == "il"):
                tch = g_in[gname]["tch"]
                lrA = cx.mp.tile([128, tch], mybir.dt.bfloat16, tag=f"{gname}_lrA", name=f"{gname}_lrA")
                lrB = cx.mp.tile([128, tch], mybir.dt.bfloat16, tag=f"{gname}_lrB", name=f"{gname}_lrB")
                vv = cx.mp.tile([128, tch], F32, tag=f"{gname}_vv", name=f"{gname}_vv")
                nc.sync.dma_start(out=lrA[:], in_=g_in[gname]["lrowA"][:])
                nc.sync.dma_start(out=lrB[:], in_=g_in[gname]["lrowB"][:])
                nc.sync.dma_start(out=vv[:], in_=g_in[gname]["val"][:])
                meta_sb[gname] = (lrA, lrB, vv)

        raw = cx.accp.tile([P, max(nw1, nw2) * D], F32, tag="raw", name="raw")

        def graph_layer(gname, prog, R, V, nwin, layer, src_table, acc, f_ag):
            lrA, lrB, vv = meta_sb[gname]
            scale = 1.0 / (layer + 2)
            emit_spmm(cx, f"{gname}L{layer}", prog, R, V, src_table[:], g_in[gname]["idx"],
                      lrA, lrB, vv, raw, scale)
            if f_ag is not None:
                f_in, f_full = f_ag
                nc.sync.dma_start(out=f_in[:].rearrange("(w p) d -> p w d", p=P),
                                  in_=raw[:, :nwin * D].rearrange("p (w d) -> p w d", w=nwin))
                nc.gpsimd.collective_compute(
                    "AllGather", ALU.bypass, replica_groups=[list(range(NCORES))],
                    ins=[f_in[:].opt()], outs=[f_full[:].opt()])
            emit_epilogue(cx, raw, acc, nwin, scale)

        if "il" in phases:
            acc_il = cx.accp.tile([P, nw1 * D], F32, tag="acc_il", name="acc_il")
            nc.sync.dma_start(
                out=acc_il[:].rearrange("p (w d) -> p w d", w=nw1),
                in_=f0_il_sh[:].rearrange("(w p) d -> p w d", p=P))
            il_f1 = ag_tiles(R1, V1, "il1")
            il_acc_ag = ag_tiles(R1, V1, "ila")
        if "bl" in phases:
            acc_bl = cx.accp.tile([P, nw2 * D], F32, tag="acc_bl", name="acc_bl")
            nc.sync.dma_start(
                out=acc_bl[:].rearrange("p (w d) -> p w d", w=nw2),
                in_=f0_bl_sh[:].rearrange("(w p) d -> p w d", p=P))
            bl_f1 = ag_tiles(R2, V2, "bl1")
            bl_acc_ag = ag_tiles(R2, V2, "bla")

        if "il" in phases:
            graph_layer("il", pp["il"], R1, V1, nw1, 0, f0_il, acc_il, il_f1)
        if "bl" in phases:
            graph_layer("bl", pp["bl"], R2, V2, nw2, 0, f0_bl, acc_bl, bl_f1)
        if "il" in phases:
            graph_layer("il", pp["il"], R1, V1, nw1, 1, il_f1[1], acc_il, None)
            acc_in, acc_il_full = il_acc_ag
            nc.sync.dma_start(out=acc_in[:].rearrange("(w p) d -> p w d", p=P),
                              in_=acc_il[:].rearrange("p (w d) -> p w d", w=nw1))
            nc.gpsimd.collective_compute(
                "AllGather", ALU.bypass, replica_groups=[list(range(NCORES))],
                ins=[acc_in[:].opt()], outs=[acc_il_full[:].opt()])
        if "bl" in phases:
            graph_layer("bl", pp["bl"], R2, V2, nw2, 1, bl_f1[1], acc_bl, None)
            acc_in2, acc_bl_full = bl_acc_ag
            nc.sync.dma_start(out=acc_in2[:].rearrange("(w p) d -> p w d", p=P),
                              in_=acc_bl[:].rearrange("p (w d) -> p w d", w=nw2))
            nc.gpsimd.collective_compute(
                "AllGather", ALU.bypass, replica_groups=[list(range(NCORES))],
                ins=[acc_in2[:].opt()], outs=[acc_bl_full[:].opt()])
        if "ag" in phases:
            lrA, lrB, vv = meta_sb["ag"]
            emit_spmm(cx, "agg", pp["ag"], RB, V1, acc_il_full[:], g_in["ag"]["idx"],
                      lrA, lrB, vv, raw, 1.0)
            ilb_in, ilb_full = ag_tiles(RB, VB, "ilb")
            nc.sync.dma_start(out=ilb_in[:].rearrange("(w p) d -> p w d", p=P),
                              in_=raw[:, :nwb * D].rearrange("p (w d) -> p w d", w=nwb))
            nc.gpsimd.collective_compute(
                "AllGather", ALU.bypass, replica_groups=[list(range(NCORES))],
                ins=[ilb_in[:].opt()], outs=[ilb_full[:].opt()])

        for p in reversed(es2):
            p.__exit__(None, None, None)
        cx.lp = pool("loss", 1)

        if debug_tables:
            for nm, t, Vv in (("dbg_acc_il", acc_il_full, V1), ("dbg_acc_bl", acc_bl_full, V2),
                              ("dbg_ilb", ilb_full, VB)):
                if t is not None:
                    o = nc.dram_tensor(nm, [Vv, D], F32, kind="ExternalOutput")
                    nc.sync.dma_start(out=o[:], in_=t[:])
                    dbg[nm] = o

        if "loss" in phases:
            bsh = BATCH // NCORES          # 256
            ng = bsh // P                  # 2 groups of my rows
            # -- load loss indices
            lidx_sb = {}
            for k, t in lidx.items():
                s = cx.lp.tile([128, t.shape[1]], I32, tag=f"li_{k}")
                nc.sync.dma_start(out=s[:], in_=t[:])
                lidx_sb[k] = s
            for k, t in aug_in.items():
                s = cx.lp.tile([128, 16], I32, tag=f"li_{k}")
                nc.sync.dma_start(out=s[:], in_=t[:])
                lidx_sb[k] = s
            # -- gathers
            def gather(tag, table, idxk, ncols):
                sb = cx.lp.tile([P, ncols * D], F32, tag=tag)
                indirect_gather_rows(cx, sb, table, lidx_sb[idxk], ncols)
                return sb
            pos_u_il = gather("pos_u_il", acc_il_full[:], "u_il", ng)
            pos_u_bl = gather("pos_u_bl", acc_bl_full[:], "u_bl", ng)
            b_il0 = gather("b_il0", ilb_full[:], "b_il0", ng)
            b_il1 = gather("b_il1", ilb_full[:], "b_il1", ng)
            b_bl0 = gather("b_bl0", acc_bl_full[:], "b_bl0", ng)
            b_bl1 = gather("b_bl1", acc_bl_full[:], "b_bl1", ng)
            aug_u = gather("aug_u", acc_bl_full[:], "aug_u_bl", 16)
            aug_b = gather("aug_b", acc_bl_full[:], "aug_b0_bl", 16)
            pos_b_il = gather("pos_b_il", ilb_full[:], "aug_b0_il", 16)  # c2 pos, full
            # -- bpr
            pr0 = cx.lp.tile([P, ng], F32, tag="pr0")
            pr1 = cx.lp.tile([P, ng], F32, tag="pr1")
            tmp = cx.lp.tile([P, ng], F32, tag="prt")
            rowdot(cx, pos_u_il, b_il0, pr0, ng, "d0")
            rowdot(cx, pos_u_bl, b_bl0, tmp, ng, "d1")
            nc.vector.tensor_add(pr0[:], pr0[:], tmp[:])
            rowdot(cx, pos_u_il, b_il1, pr1, ng, "d2")
            rowdot(cx, pos_u_bl, b_bl1, tmp, ng, "d3")
            nc.vector.tensor_add(pr1[:], pr1[:], tmp[:])
            x = cx.lp.tile([P, ng], F32, tag="bprx")
            nc.vector.tensor_tensor(out=x[:], in0=pr1[:], in1=pr0[:], op=ALU.subtract)
            # softplus(x) = relu(x) + ln(1 + exp(-|x|))  (Softplus LUT unavailable)
            negx = cx.lp.tile([P, ng], F32, tag="bprnx")
            nc.vector.tensor_scalar_mul(negx[:], x[:], -1.0)
            nax = cx.lp.tile([P, ng], F32, tag="bprax")
            nc.vector.tensor_tensor(out=nax[:], in0=x[:], in1=negx[:], op=ALU.min)
            e = cx.lp.tile([P, ng], F32, tag="bpre")
            nc.scalar.activation(e[:], nax[:], AF.Exp)
            nc.vector.tensor_scalar_add(e[:], e[:], 1.0)
            l1p = cx.lp.tile([P, ng], F32, tag="bprl")
            nc.scalar.activation(l1p[:], e[:], AF.Ln)
            sp = cx.lp.tile([P, ng], F32, tag="bprsp")
            nc.vector.tensor_scalar_max(sp[:], x[:], 0.0)
            nc.vector.tensor_add(sp[:], sp[:], l1p[:])

            # -- contrastive: c1 = closs(IL_u[users], BL_u[users]) with my pos rows;
            #    c2 = closs(IL_b[b0], BL_b[b0]).
            # normalize (full aug tables; my pos slices)
            normalize_rows(cx, aug_u, 16, "nau")
            normalize_rows(cx, aug_b, 16, "nab")
            normalize_rows(cx, pos_u_il, ng, "npu")
            # c2 pos rows (my slice of pos_b_il): normalize only my cols
            my_pos_b = cx.lp.tile([P, ng * D], F32, tag="my_pb")
            # my rows of the full b0 tables sit at groups [2*core .. 2*core+2) --
            # but core id differs per core! Instead gather my slice separately:
            # reuse b_il0 (= IL_b[bundles[my,0]]) normalized.
            nc.vector.tensor_copy(my_pos_b[:], b_il0[:, :ng * D])
            normalize_rows(cx, my_pos_b, ng, "npb")

            part = cx.lp.tile([P, 4], F32, tag="parts")  # [bpr, c1, c2, unused]
            nc.vector.memset(part[:], 0.0)
            nc.vector.reduce_sum(part[:, 0:1], sp[:].rearrange("p (w d) -> p w d", w=1),
                                 axis=mybir.AxisListType.X)

            def closs_partial(pos_my, aug_full, aug_my_cols, out_col):
                # pos_my [128, ng*64] normalized; aug_full [128, 16*64] normalized
                posT = transpose_groups(cx, pos_my, ng, "pT")      # [64, ng*128]
                augT = transpose_groups(cx, aug_full, 16, "aT")    # [64, 16*128]
                ps = cx.lp.tile([P, ng], F32, tag="ps")
                # pos_score rows: aug rows aligned with my pos = my core's slice of aug
                rowdot(cx, pos_my, aug_my_cols, ps, ng, f"psd{out_col}")
                lse = cx.lp.tile([P, ng], F32, tag="lse")
                for g in range(ng):
                    ttl_ps = cx.psp.tile([P, 512], F32, space="PSUM", tag="ttl", bufs=1)
                    ttl = cx.lp.tile([P, BATCH], F32, tag="ttl")
                    for nb_ in range(BATCH // 512):
                        nc.tensor.matmul(
                            out=ttl_ps[:, :512],
                            lhsT=posT[:D, g * P:(g + 1) * P],
                            rhs=augT[:D, nb_ * 512:(nb_ + 1) * 512],
                            start=True, stop=True)
                        nc.vector.tensor_copy(ttl[:, nb_ * 512:(nb_ + 1) * 512], ttl_ps[:, :512])
                    mx = cx.lp.tile([P, 1], F32, tag="mx")
                    nc.vector.reduce_max(mx[:], ttl[:].rearrange("p (w d) -> p w d", w=1),
                                         axis=mybir.AxisListType.X)
                    nmx = cx.lp.tile([P, 1], F32, tag="nmx")
                    nc.vector.tensor_scalar_mul(nmx[:], mx[:], -4.0)
                    ex = cx.lp.tile([P, BATCH], F32, tag="ex")
                    se = cx.lp.tile([P, 1], F32, tag="se")
                    nc.scalar.activation(ex[:], ttl[:], AF.Exp, bias=nmx[:, :1], scale=4.0,
                                         accum_out=se[:, :1])
                    ln = cx.lp.tile([P, 1], F32, tag="ln")
                    nc.scalar.activation(ln[:], se[:], AF.Ln)
                    # lse_g = ln + 4*mx
                    m4 = cx.lp.tile([P, 1], F32, tag="m4")
                    nc.vector.tensor_scalar_mul(m4[:], mx[:], 4.0)
                    nc.vector.tensor_add(lse[:, g:g + 1], ln[:], m4[:])
                # partial = sum_rows(4*ps - lse)
                t4 = cx.lp.tile([P, ng], F32, tag="t4")
                nc.vector.tensor_scalar_mul(t4[:], ps[:], 4.0)
                nc.vector.tensor_tensor(out=t4[:], in0=t4[:], in1=lse[:], op=ALU.subtract)
                nc.vector.reduce_sum(part[:, out_col:out_col + 1],
                                     t4[:].rearrange("p (w d) -> p w d", w=1),
                                     axis=mybir.AxisListType.X)

            # aug_my_cols: my core's slice of the aug tables — per-core col offset...
            # per-core differences must come from inputs: gather my aug rows separately.
            aug_u_my = gather("aug_u_my", acc_bl_full[:], "u_bl", ng)
            normalize_rows(cx, aug_u_my, ng, "naum")
            aug_b_my = gather("aug_b_my", acc_bl_full[:], "b_bl0", ng)
            normalize_rows(cx, aug_b_my, ng, "nabm")
            closs_partial(pos_u_il, aug_u, aug_u_my, 1)
            closs_partial(my_pos_b, aug_b, aug_b_my, 2)

            # -- cross-partition sum of partials via ones-matmul
            pp_ps = cx.psp.tile([P, 4], F32, space="PSUM", tag="ppps", bufs=1)
            nc.tensor.matmul(out=pp_ps[:1, :4], lhsT=ones_col[:], rhs=part[:],
                             start=True, stop=True)
            psum_sb = cx.lp.tile([1, 4], F32, tag="psums")
            nc.vector.tensor_copy(psum_sb[:], pp_ps[:1, :4])
            ar_in = cx.dramp.tile([1, 4], F32, tag="ar_in")
            ar_out = cx.dramp.tile([1, 4], F32, addr_space="Shared", tag="ar_out")
            nc.sync.dma_start(out=ar_in[:], in_=psum_sb[:])
            nc.gpsimd.collective_compute(
                "AllReduce", ALU.add, replica_groups=[list(range(NCORES))],
                ins=[ar_in[:].opt()], outs=[ar_out[:].opt()])
            fin = cx.lp.tile([1, 4], F32, tag="fin")
            nc.sync.dma_start(out=fin[:], in_=ar_out[:])
            res = cx.lp.tile([1, 2], F32, tag="res")
            nc.vector.tensor_scalar_mul(res[:, 0:1], fin[:, 0:1], 1.0 / BATCH)
            t = cx.lp.tile([1, 1], F32, tag="rt")
            nc.vector.tensor_add(t[:], fin[:, 1:2], fin[:, 2:3])
            nc.vector.tensor_scalar_mul(res[:, 1:2], t[:], -0.5 / BATCH)
            nc.sync.dma_start(out=out_t[:], in_=res[:])
        else:
            z = cx.lp.tile([1, 2], F32, tag="z")
            nc.vector.memset(z[:], 0.0)
            nc.sync.dma_start(out=out_t[:], in_=z[:])

        for p in reversed(es):
            p.__exit__(None, None, None)
    nc.compile()
    return nc, dbg


# ---------------------------------------------------------------- entry point

def _install_ntff_hook():
    if "antenv.axon_hooks" in sys.modules:
        return
    try:
        mod = types.ModuleType("antenv.axon_hooks")
        _hook = [None]
        mod.set_axon_ntff_profile_hook = lambda h: _hook.__setitem__(0, h)
        mod.get_axon_ntff_profile_hook = lambda: _hook[0]
        sys.modules["antenv.axon_hooks"] = mod
        import antenv
        antenv.axon_hooks = mod
        from trn_agent_boot.trn_boot import _ntff_profile_via_ctypes
        hook = _ntff_profile_via_ctypes("/opt/axon/libaxon_pjrt.so")
        if hook is not None:
            mod.set_axon_ntff_profile_hook(hook)
    except Exception:
        pass


def make_in_maps(pp):
    maps = []
    for c in range(NCORES):
        R1, V1, R2, V2, RB, VB = pp["dims"]
        m = {
            "f0_il": pp["f0_il"], "f0_bl": pp["f0_bl"],
            "f0_il_sh": pp["f0_il"][c * R1:(c + 1) * R1],
            "f0_bl_sh": pp["f0_bl"][c * R2:(c + 1) * R2],
            "aug_u_bl": pp["aug_u_bl"], "aug_b0_bl": pp["aug_b0_bl"],
            "aug_b0_il": pp["aug_b0_il"],
        }
        import ml_dtypes
        for gname, key in (("il", "il"), ("bl", "bl"), ("ag", "ag")):
            idxs, lrowA, lrowB, vals, program, tch = pp[key]
            m[f"{gname}_idx"] = wrap_idx16(idxs[c])
            m[f"{gname}_lrowA"] = np.ascontiguousarray(lrowA[c].reshape(-1, P).T).astype(ml_dtypes.bfloat16)
            m[f"{gname}_lrowB"] = np.ascontiguousarray(lrowB[c].reshape(-1, P).T).astype(ml_dtypes.bfloat16)
            m[f"{gname}_val"] = np.ascontiguousarray(vals[c].reshape(-1, P).T)
        for k, v in pp["loss"][c].items():
            m[f"loss_{k}"] = v
        maps.append(m)
    return maps


_CACHE = {}


def kernel(**inputs) -> np.ndarray:
    _install_ntff_hook()
    pp = preprocess(inputs)
    key = "full"
    if key not in _CACHE:
        _CACHE[key] = build(pp)
    nc, dbg = _CACHE[key]
    in_maps = make_in_maps(pp)
    trace = bool(int(os.environ.get("DSCBR_TRACE", "0")))
    res = run_bass_kernel_spmd(nc, in_maps, core_ids=list(range(NCORES)), trace=trace)
    if trace and res.exec_time_ns:
        print(f"HW exec time: {res.exec_time_ns} ns")
    out = res.results[0]["out"].reshape(2).astype(np.float32)
    return out

